# revision 5
# baseline (speedup 1.0000x reference)
"""Trainium2 Bass kernel: column-parallel linear  out = input_ @ weight.T + bias.

Problem shapes (hardcoded):
    input_: [4096, 2, 4096] f32  (S, B, H)
    weight: [16384, 4096]   f32  (F, H)
    bias:   [16384]         f32
    out:    [4096, 2, 16384] f32

Strategy: tensor-parallel over the output dim F. Each of the 8 cores gets the
full input and a 2048-row slice of the weight; it computes out[:, :, c*2048:
(c+1)*2048] locally (no collectives). The host pre-transposes both operands so
the contraction dim H lands on SBUF partitions with natural (contiguous) DMAs,
and concatenates the 8 output shards at the end.

Device kernel per core: out[m, f] = sum_h XT[h, m] * WT[h, f] + bias[f]
  - lhsT (stationary) = XT tile [128h, 128m], rhs (moving) = WT [128h, 512f]
  - float32r matmuls (fp32 truncated to FP22, fp32 accumulate): 1 cycle/row
  - WT half [4096, 1024] resident in SBUF; XT streamed once per half
  - bias added during PSUM->SBUF copyback on the vector engine
"""

import os
import sys

import numpy as np

for _p in ("/opt/trn_rl_repo", "/root/.axon_site/_ro/trn_rl_repo"):
    if os.path.isdir(_p) and _p not in sys.path:
        sys.path.insert(0, _p)

P = 128
FCHUNK = 512  # one PSUM bank of fp32
S, B, H, F = 4096, 2, 4096, 16384
N_CORES = 8
M = S * B
FS = F // N_CORES


def build_nc(H=H, M=M, FS=FS, f_half=1024):
    from concourse import bacc
    import concourse.mybir as mybir
    import concourse.tile as tile

    KT, MT = H // P, M // P
    F_HALF = min(f_half, FS)
    N_HALF = FS // F_HALF
    CHUNKS = max(1, F_HALF // FCHUNK)
    FC = min(FCHUNK, F_HALF)

    f32 = mybir.dt.float32
    f32r = mybir.dt.float32r

    nc = bacc.Bacc(None, target_bir_lowering=False)
    xt = nc.declare_dram_parameter("xt", [H, M], f32r, isOutput=False)
    wt = nc.declare_dram_parameter("wt", [H, FS], f32r, isOutput=False)
    bias = nc.declare_dram_parameter("bias", [P, FS], f32, isOutput=False)
    out = nc.declare_dram_parameter("out", [M, FS], f32, isOutput=True)

    xt_r = xt[:, :].rearrange("(kt p) m -> p kt m", p=P)
    wt_r = wt[:, :].rearrange("(kt p) f -> p kt f", p=P)

    with tile.TileContext(nc) as tc:
        with (
            tc.tile_pool(name="wpool", bufs=1) as wpool,
            tc.tile_pool(name="xpool", bufs=3) as xpool,
            tc.tile_pool(name="opool", bufs=3) as opool,
            tc.tile_pool(name="bpool", bufs=1) as bpool,
            tc.tile_pool(name="psum", bufs=4, space="PSUM") as pspool,
        ):
            bias_sb = bpool.tile([P, FS], f32)
            nc.sync.dma_start(out=bias_sb[:, :], in_=bias[:, :])

            for fh in range(N_HALF):
                f0 = fh * F_HALF
                w_half = wpool.tile([P, KT, F_HALF], f32r, tag="whalf")
                nc.sync.dma_start(
                    out=w_half[:, :, :], in_=wt_r[:, :, f0 : f0 + F_HALF]
                )
                for mt in range(MT):
                    m0 = mt * P
                    x_tile = xpool.tile([P, KT, P], f32r, tag="xtile")
                    nc.sync.dma_start(
                        out=x_tile[:, :, :], in_=xt_r[:, :, m0 : m0 + P]
                    )
                    o_tile = opool.tile([P, F_HALF], f32, tag="otile")
                    for fc in range(CHUNKS):
                        ps = pspool.tile([P, FC], f32, tag="ps")
                        for kt in range(KT):
                            nc.tensor.matmul(
                                ps[:, :],
                                lhsT=x_tile[:, kt, :],
                                rhs=w_half[:, kt, fc * FC : (fc + 1) * FC],
                                start=(kt == 0),
                                stop=(kt == KT - 1),
                            )
                        nc.vector.tensor_add(
                            o_tile[:, fc * FC : (fc + 1) * FC],
                            ps[:, :],
                            bias_sb[:, f0 + fc * FC : f0 + (fc + 1) * FC],
                        )
                    nc.sync.dma_start(
                        out=out[m0 : m0 + P, f0 : f0 + F_HALF], in_=o_tile[:, :]
                    )
    nc.compile()
    return nc


def make_in_maps(input_, weight, bias):
    X = np.ascontiguousarray(np.asarray(input_, dtype=np.float32).reshape(M, H))
    XT = np.ascontiguousarray(X.T)
    W = np.asarray(weight, dtype=np.float32)
    b = np.asarray(bias, dtype=np.float32)
    in_maps = []
    for c in range(N_CORES):
        WTc = np.ascontiguousarray(W[c * FS : (c + 1) * FS].T)
        bc = np.ascontiguousarray(
            np.broadcast_to(b[c * FS : (c + 1) * FS][None, :], (P, FS))
        )
        in_maps.append({"xt": XT, "wt": WTc, "bias": bc})
    return in_maps


_NC_CACHE = {}


def run_spmd(input_, weight, bias, trace=False, **kw):
    from concourse.bass_utils import run_bass_kernel_spmd

    if "full" not in _NC_CACHE:
        _NC_CACHE["full"] = build_nc()
    nc = _NC_CACHE["full"]
    in_maps = make_in_maps(input_, weight, bias)
    res = run_bass_kernel_spmd(
        nc, in_maps, core_ids=list(range(N_CORES)), trace=trace, **kw
    )
    outs = [np.asarray(res.results[c]["out"]) for c in range(N_CORES)]
    full = np.concatenate(outs, axis=1).reshape(S, B, F)
    return full, res


def kernel(input_, weight, bias):
    out, _ = run_spmd(input_, weight, bias, trace=False)
    return out


# revision 7
# speedup vs baseline: 1.0295x; 1.0295x over previous
"""Trainium2 Bass kernel: column-parallel linear  out = input_ @ weight.T + bias.

Problem shapes (hardcoded):
    input_: [4096, 2, 4096] f32  (S, B, H)
    weight: [16384, 4096]   f32  (F, H)
    bias:   [16384]         f32
    out:    [4096, 2, 16384] f32

Tensor-parallel over the output dim F: each of the 8 cores gets the full input
and a 2048-row slice of the weight, computing its output slice locally (no
collectives). The host pre-permutes both operands into the exact SBUF tile
layouts (contraction dim H on partitions, 16KB+ contiguous per partition) so
every DMA is a large contiguous burst, and concatenates the shards at the end.

Device kernel per core: out[m, f] = sum_h XT[h, m] * WT[h, f] + bias[f]
  - lhsT (stationary) = XT tile [128h, 128m], rhs (moving) = WT [128h, 512f]
  - float32r matmuls (fp32 read truncated to FP22, fp32 accumulate): 1 cyc/row
  - W half [4096, 1024] resident in SBUF as 32 per-kt tiles; X streamed once
    per half; x-loads on SP HWDGE ring, w-loads on Act ring, out via SWDGE
  - kt-outer/fc-inner so both psum chunks reuse one stationary load
  - bias added during PSUM->SBUF copyback on the vector engine
"""

import os
import sys

import numpy as np

for _p in ("/opt/trn_rl_repo", "/root/.axon_site/_ro/trn_rl_repo"):
    if os.path.isdir(_p) and _p not in sys.path:
        sys.path.insert(0, _p)

P = 128
FCHUNK = 512  # one PSUM bank of fp32
S, B, H, F = 4096, 2, 4096, 16384
N_CORES = 8
M = S * B
FS = F // N_CORES


def build_nc(H=H, M=M, FS=FS, f_half=1024, w_bufs=34):
    from concourse import bacc
    import concourse.mybir as mybir
    import concourse.tile as tile

    KT, MT = H // P, M // P
    F_HALF = min(f_half, FS)
    N_HALF = FS // F_HALF
    FC = min(FCHUNK, F_HALF)
    CHUNKS = F_HALF // FC
    w_bufs = min(w_bufs, N_HALF * KT)

    f32 = mybir.dt.float32
    f32r = mybir.dt.float32r

    nc = bacc.Bacc(None, target_bir_lowering=False)
    # Pre-tiled layouts (host produces these):
    #   xt[mt, p, kt*P + mi] = input[mt*P + mi, kt*P + p]
    #   wt[fh, p, kt*F_HALF + fj] = weight_shard[fh*F_HALF + fj, kt*P + p]
    xt = nc.declare_dram_parameter("xt", [MT, P, KT * P], f32r, isOutput=False)
    wt = nc.declare_dram_parameter("wt", [N_HALF, P, KT * F_HALF], f32r, isOutput=False)
    bias = nc.declare_dram_parameter("bias", [P, FS], f32, isOutput=False)
    out = nc.declare_dram_parameter("out", [M, FS], f32, isOutput=True)

    with tile.TileContext(nc) as tc:
        with (
            tc.tile_pool(name="wpool", bufs=w_bufs) as wpool,
            tc.tile_pool(name="xpool", bufs=3) as xpool,
            tc.tile_pool(name="opool", bufs=3) as opool,
            tc.tile_pool(name="bpool", bufs=1) as bpool,
            tc.tile_pool(name="psum", bufs=8, space="PSUM") as pspool,
        ):
            bias_sb = bpool.tile([P, FS], f32)
            nc.scalar.dma_start(out=bias_sb[:, :], in_=bias[:, :])

            for fh in range(N_HALF):
                f0 = fh * F_HALF
                w_kt = []
                for kt in range(KT):
                    wk = wpool.tile([P, F_HALF], f32r, tag="wkt")
                    nc.scalar.dma_start(
                        out=wk[:, :],
                        in_=wt[fh, :, kt * F_HALF : (kt + 1) * F_HALF],
                    )
                    w_kt.append(wk)
                for mt in range(MT):
                    m0 = mt * P
                    x_tile = xpool.tile([P, KT * P], f32r, tag="xtile")
                    nc.sync.dma_start(out=x_tile[:, :], in_=xt[mt, :, :])
                    o_tile = opool.tile([P, F_HALF], f32, tag="otile")
                    ps = [
                        pspool.tile([P, FC], f32, tag="ps", name=f"ps{fc}")
                        for fc in range(CHUNKS)
                    ]
                    for kt in range(KT):
                        lhsT = x_tile[:, kt * P : (kt + 1) * P]
                        for fc in range(CHUNKS):
                            nc.tensor.matmul(
                                ps[fc][:, :],
                                lhsT=lhsT,
                                rhs=w_kt[kt][:, fc * FC : (fc + 1) * FC],
                                start=(kt == 0),
                                stop=(kt == KT - 1),
                            )
                    for fc in range(CHUNKS):
                        nc.vector.tensor_add(
                            o_tile[:, fc * FC : (fc + 1) * FC],
                            ps[fc][:, :],
                            bias_sb[:, f0 + fc * FC : f0 + (fc + 1) * FC],
                        )
                    nc.gpsimd.dma_start(
                        out=out[m0 : m0 + P, f0 : f0 + F_HALF], in_=o_tile[:, :]
                    )
    nc.compile()
    return nc


def make_in_maps(input_, weight, bias, f_half=1024):
    KT, MT = H // P, M // P
    F_HALF = min(f_half, FS)
    N_HALF = FS // F_HALF
    X = np.asarray(input_, dtype=np.float32).reshape(M, H)
    # xt[mt, p, kt, mi] = X[mt*P+mi, kt*P+p]
    XTt = np.ascontiguousarray(
        X.reshape(MT, P, KT, P).transpose(0, 3, 2, 1).reshape(MT, P, KT * P)
    )
    W = np.asarray(weight, dtype=np.float32)
    b = np.asarray(bias, dtype=np.float32)
    in_maps = []
    for c in range(N_CORES):
        Wc = W[c * FS : (c + 1) * FS]  # [FS, H]
        # wt[fh, p, kt, fj] = Wc[fh*F_HALF+fj, kt*P+p]
        WTc = np.ascontiguousarray(
            Wc.reshape(N_HALF, F_HALF, KT, P)
            .transpose(0, 3, 2, 1)
            .reshape(N_HALF, P, KT * F_HALF)
        )
        bc = np.ascontiguousarray(
            np.broadcast_to(b[c * FS : (c + 1) * FS][None, :], (P, FS))
        )
        in_maps.append({"xt": XTt, "wt": WTc, "bias": bc})
    return in_maps


_NC_CACHE = {}


def run_spmd(input_, weight, bias, trace=False, **kw):
    from concourse.bass_utils import run_bass_kernel_spmd

    if "full" not in _NC_CACHE:
        _NC_CACHE["full"] = build_nc()
    nc = _NC_CACHE["full"]
    in_maps = make_in_maps(input_, weight, bias)
    res = run_bass_kernel_spmd(
        nc, in_maps, core_ids=list(range(N_CORES)), trace=trace, **kw
    )
    outs = [np.asarray(res.results[c]["out"]) for c in range(N_CORES)]
    full = np.concatenate(outs, axis=1).reshape(S, B, F)
    return full, res


def kernel(input_, weight, bias):
    out, _ = run_spmd(input_, weight, bias, trace=False)
    return out


# revision 8
# speedup vs baseline: 1.2292x; 1.1940x over previous
"""Trainium2 Bass kernel: column-parallel linear  out = input_ @ weight.T + bias.

Problem shapes (hardcoded):
    input_: [4096, 2, 4096] f32  (S, B, H)
    weight: [16384, 4096]   f32  (F, H)
    bias:   [16384]         f32
    out:    [4096, 2, 16384] f32

Tensor-parallel over the output dim F: each of the 8 cores gets the full input
and a 2048-row slice of the weight, computing its output slice locally (no
collectives). The host pre-permutes both operands into the exact SBUF tile
layouts (contraction dim H on partitions, 16KB+ contiguous per partition) so
every DMA is a large contiguous burst, and concatenates the shards at the end.

Device kernel per core: out[m, f] = sum_h XT[h, m] * WT[h, f] + bias[f]
  - lhsT (stationary) = XT tile [128h, 128m], rhs (moving) = WT [128h, 512f]
  - float32r matmuls (fp32 read truncated to FP22, fp32 accumulate): 1 cyc/row
  - W half [4096, 1024] resident in SBUF as 32 per-kt tiles; X streamed once
    per half; x-loads on SP HWDGE ring, w-loads on Act ring, out via SWDGE
  - kt-outer/fc-inner so both psum chunks reuse one stationary load
  - bias added during PSUM->SBUF copyback on the vector engine
"""

import os
import sys

import numpy as np

for _p in ("/opt/trn_rl_repo", "/root/.axon_site/_ro/trn_rl_repo"):
    if os.path.isdir(_p) and _p not in sys.path:
        sys.path.insert(0, _p)

P = 128
FCHUNK = 512  # one PSUM bank of fp32
S, B, H, F = 4096, 2, 4096, 16384
N_CORES = 8
M = S * B
FS = F // N_CORES


def build_nc(H=H, M=M, FS=FS, f_half=1024, w_bufs=34):
    from concourse import bacc
    import concourse.mybir as mybir
    import concourse.tile as tile

    KT, MT = H // P, M // P
    F_HALF = min(f_half, FS)
    N_HALF = FS // F_HALF
    FC = min(FCHUNK, F_HALF)
    CHUNKS = F_HALF // FC
    w_bufs = min(w_bufs, N_HALF * KT)

    f32 = mybir.dt.float32
    f32r = mybir.dt.float32r

    nc = bacc.Bacc(None, target_bir_lowering=False)
    # Pre-tiled layouts (host produces these):
    #   xt[mt, p, kt*P + mi] = input[mt*P + mi, kt*P + p]
    #   wt[fh, p, kt*F_HALF + fj] = weight_shard[fh*F_HALF + fj, kt*P + p]
    xt = nc.declare_dram_parameter("xt", [MT, P, KT * P], f32r, isOutput=False)
    wt = nc.declare_dram_parameter("wt", [N_HALF, P, KT * F_HALF], f32r, isOutput=False)
    bias = nc.declare_dram_parameter("bias", [P, FS], f32, isOutput=False)
    out = nc.declare_dram_parameter("out", [M, FS], f32, isOutput=True)

    with tile.TileContext(nc) as tc:
        with (
            tc.tile_pool(name="wpool", bufs=w_bufs) as wpool,
            tc.tile_pool(name="xpool", bufs=3) as xpool,
            tc.tile_pool(name="opool", bufs=3) as opool,
            tc.tile_pool(name="bpool", bufs=1) as bpool,
            tc.tile_pool(name="psum", bufs=8, space="PSUM") as pspool,
        ):
            bias_sb = bpool.tile([P, FS], f32)
            nc.scalar.dma_start(out=bias_sb[:, :], in_=bias[:, :])

            for fh in range(N_HALF):
                f0 = fh * F_HALF
                w_kt = []
                for kt in range(KT):
                    wk = wpool.tile([P, F_HALF], f32r, tag="wkt")
                    nc.scalar.dma_start(
                        out=wk[:, :],
                        in_=wt[fh, :, kt * F_HALF : (kt + 1) * F_HALF],
                    )
                    w_kt.append(wk)
                for mt in range(MT):
                    m0 = mt * P
                    x_tile = xpool.tile([P, KT * P], f32r, tag="xtile")
                    nc.sync.dma_start(out=x_tile[:, :], in_=xt[mt, :, :])
                    o_tile = opool.tile([P, F_HALF], f32, tag="otile")
                    for fc in range(CHUNKS):
                        ps = pspool.tile([P, FC], f32, tag="ps")
                        for kt in range(KT):
                            nc.tensor.matmul(
                                ps[:, :],
                                lhsT=x_tile[:, kt * P : (kt + 1) * P],
                                rhs=w_kt[kt][:, fc * FC : (fc + 1) * FC],
                                start=(kt == 0),
                                stop=(kt == KT - 1),
                            )
                        nc.vector.tensor_add(
                            o_tile[:, fc * FC : (fc + 1) * FC],
                            ps[:, :],
                            bias_sb[:, f0 + fc * FC : f0 + (fc + 1) * FC],
                        )
                    nc.gpsimd.dma_start(
                        out=out[m0 : m0 + P, f0 : f0 + F_HALF], in_=o_tile[:, :]
                    )
    nc.compile()
    return nc


def make_in_maps(input_, weight, bias, f_half=1024):
    KT, MT = H // P, M // P
    F_HALF = min(f_half, FS)
    N_HALF = FS // F_HALF
    X = np.asarray(input_, dtype=np.float32).reshape(M, H)
    # xt[mt, p, kt, mi] = X[mt*P+mi, kt*P+p]
    XTt = np.ascontiguousarray(
        X.reshape(MT, P, KT, P).transpose(0, 3, 2, 1).reshape(MT, P, KT * P)
    )
    W = np.asarray(weight, dtype=np.float32)
    b = np.asarray(bias, dtype=np.float32)
    in_maps = []
    for c in range(N_CORES):
        Wc = W[c * FS : (c + 1) * FS]  # [FS, H]
        # wt[fh, p, kt, fj] = Wc[fh*F_HALF+fj, kt*P+p]
        WTc = np.ascontiguousarray(
            Wc.reshape(N_HALF, F_HALF, KT, P)
            .transpose(0, 3, 2, 1)
            .reshape(N_HALF, P, KT * F_HALF)
        )
        bc = np.ascontiguousarray(
            np.broadcast_to(b[c * FS : (c + 1) * FS][None, :], (P, FS))
        )
        in_maps.append({"xt": XTt, "wt": WTc, "bias": bc})
    return in_maps


_NC_CACHE = {}


def run_spmd(input_, weight, bias, trace=False, **kw):
    from concourse.bass_utils import run_bass_kernel_spmd

    if "full" not in _NC_CACHE:
        _NC_CACHE["full"] = build_nc()
    nc = _NC_CACHE["full"]
    in_maps = make_in_maps(input_, weight, bias)
    res = run_bass_kernel_spmd(
        nc, in_maps, core_ids=list(range(N_CORES)), trace=trace, **kw
    )
    outs = [np.asarray(res.results[c]["out"]) for c in range(N_CORES)]
    full = np.concatenate(outs, axis=1).reshape(S, B, F)
    return full, res


def kernel(input_, weight, bias):
    out, _ = run_spmd(input_, weight, bias, trace=False)
    return out


# revision 11
# speedup vs baseline: 1.3169x; 1.0714x over previous
"""Trainium2 Bass kernel: column-parallel linear  out = input_ @ weight.T + bias.

Problem shapes (hardcoded):
    input_: [4096, 2, 4096] f32  (S, B, H)
    weight: [16384, 4096]   f32  (F, H)
    bias:   [16384]         f32
    out:    [4096, 2, 16384] f32

Tensor-parallel over the output dim F: each of the 8 cores gets the full input
and a 2048-row slice of the weight, computing its output slice locally (no
collectives). The host pre-permutes both operands into the exact SBUF tile
layouts (contraction dim H on partitions, 16KB+ contiguous per partition) so
every DMA is a large contiguous burst, and concatenates the shards at the end.

Device kernel per core: out[m, f] = sum_h XT[h, m] * WT[h, f] + bias[f]
  - lhsT (stationary) = XT tile [128h, 128m], rhs (moving) = WT [128h, 512f]
  - float32r matmuls (fp32 read truncated to FP22, fp32 accumulate): 1 cyc/row
  - W half [4096, 1024] resident in SBUF as 32 per-kt tiles; X streamed once
    per half; x-loads on SP HWDGE ring, w-loads on Act ring, out via SWDGE
  - kt-outer/fc-inner so both psum chunks reuse one stationary load
  - bias added during PSUM->SBUF copyback on the vector engine
"""

import os
import sys

import numpy as np

for _p in ("/opt/trn_rl_repo", "/root/.axon_site/_ro/trn_rl_repo"):
    if os.path.isdir(_p) and _p not in sys.path:
        sys.path.insert(0, _p)

P = 128
FCHUNK = 512  # one PSUM bank of fp32
S, B, H, F = 4096, 2, 4096, 16384
N_CORES = 8
M = S * B
FS = F // N_CORES


def build_nc(H=H, M=M, FS=FS, f_half=1024, w_bufs=34, pair_fc=False):
    from concourse import bacc
    import concourse.mybir as mybir
    import concourse.tile as tile

    KT, MT = H // P, M // P
    F_HALF = min(f_half, FS)
    N_HALF = FS // F_HALF
    FC = min(FCHUNK, F_HALF)
    CHUNKS = F_HALF // FC
    w_bufs = min(w_bufs, N_HALF * KT)

    f32 = mybir.dt.float32
    f32r = mybir.dt.float32r

    nc = bacc.Bacc(None, target_bir_lowering=False)
    # Pre-tiled layouts (host produces these):
    #   xt[mt, p, kt*P + mi] = input[mt*P + mi, kt*P + p]
    #   wt[fh, p, kt*F_HALF + fj] = weight_shard[fh*F_HALF + fj, kt*P + p]
    xt = nc.declare_dram_parameter("xt", [MT, P, KT * P], f32r, isOutput=False)
    wt = nc.declare_dram_parameter("wt", [N_HALF, P, KT * F_HALF], f32r, isOutput=False)
    bias = nc.declare_dram_parameter("bias", [P, FS], f32, isOutput=False)
    out = nc.declare_dram_parameter("out", [M, FS], f32, isOutput=True)

    with tile.TileContext(nc) as tc:
        with (
            tc.tile_pool(name="wpool", bufs=w_bufs) as wpool,
            tc.tile_pool(name="xpool", bufs=3) as xpool,
            tc.tile_pool(name="opool", bufs=3) as opool,
            tc.tile_pool(name="bpool", bufs=1) as bpool,
            tc.tile_pool(name="psum", bufs=8, space="PSUM") as pspool,
        ):
            bias_sb = bpool.tile([P, FS], f32)
            nc.scalar.dma_start(out=bias_sb[:, :], in_=bias[:, :])

            for fh in range(N_HALF):
                f0 = fh * F_HALF
                w_kt = []
                for kt in range(KT):
                    wk = wpool.tile([P, F_HALF], f32r, tag="wkt")
                    nc.scalar.dma_start(
                        out=wk[:, :],
                        in_=wt[fh, :, kt * F_HALF : (kt + 1) * F_HALF],
                    )
                    w_kt.append(wk)
                for mt in range(MT):
                    m0 = mt * P
                    x_tile = xpool.tile([P, KT * P], f32r, tag="xtile")
                    nc.sync.dma_start(out=x_tile[:, :], in_=xt[mt, :, :])
                    o_tile = opool.tile([P, F_HALF], f32, tag="otile")
                    if pair_fc:
                        ps = [
                            pspool.tile([P, FC], f32, tag="ps", name=f"ps{_fc}")
                            for _fc in range(CHUNKS)
                        ]
                        for kt in range(KT):
                            for fc in range(CHUNKS):
                                nc.tensor.matmul(
                                    ps[fc][:, :],
                                    lhsT=x_tile[:, kt * P : (kt + 1) * P],
                                    rhs=w_kt[kt][:, fc * FC : (fc + 1) * FC],
                                    start=(kt == 0),
                                    stop=(kt == KT - 1),
                                )
                        for fc in range(CHUNKS):
                            nc.vector.tensor_add(
                                o_tile[:, fc * FC : (fc + 1) * FC],
                                ps[fc][:, :],
                                bias_sb[:, f0 + fc * FC : f0 + (fc + 1) * FC],
                            )
                    else:
                        for fc in range(CHUNKS):
                            ps = pspool.tile([P, FC], f32, tag="ps")
                            for kt in range(KT):
                                nc.tensor.matmul(
                                    ps[:, :],
                                    lhsT=x_tile[:, kt * P : (kt + 1) * P],
                                    rhs=w_kt[kt][:, fc * FC : (fc + 1) * FC],
                                    start=(kt == 0),
                                    stop=(kt == KT - 1),
                                )
                            nc.vector.tensor_add(
                                o_tile[:, fc * FC : (fc + 1) * FC],
                                ps[:, :],
                                bias_sb[:, f0 + fc * FC : f0 + (fc + 1) * FC],
                            )
                    nc.gpsimd.dma_start(
                        out=out[m0 : m0 + P, f0 : f0 + F_HALF], in_=o_tile[:, :]
                    )
    nc.compile()
    return nc


def make_in_maps(input_, weight, bias, f_half=1024):
    KT, MT = H // P, M // P
    F_HALF = min(f_half, FS)
    N_HALF = FS // F_HALF
    X = np.asarray(input_, dtype=np.float32).reshape(M, H)
    # xt[mt, p, kt, mi] = X[mt*P+mi, kt*P+p]
    XTt = np.ascontiguousarray(
        X.reshape(MT, P, KT, P).transpose(0, 3, 2, 1).reshape(MT, P, KT * P)
    )
    W = np.asarray(weight, dtype=np.float32)
    b = np.asarray(bias, dtype=np.float32)
    in_maps = []
    for c in range(N_CORES):
        Wc = W[c * FS : (c + 1) * FS]  # [FS, H]
        # wt[fh, p, kt, fj] = Wc[fh*F_HALF+fj, kt*P+p]
        WTc = np.ascontiguousarray(
            Wc.reshape(N_HALF, F_HALF, KT, P)
            .transpose(0, 3, 2, 1)
            .reshape(N_HALF, P, KT * F_HALF)
        )
        bc = np.ascontiguousarray(
            np.broadcast_to(b[c * FS : (c + 1) * FS][None, :], (P, FS))
        )
        in_maps.append({"xt": XTt, "wt": WTc, "bias": bc})
    return in_maps


_NC_CACHE = {}


def run_spmd(input_, weight, bias, trace=False, **kw):
    from concourse.bass_utils import run_bass_kernel_spmd

    if "full" not in _NC_CACHE:
        _NC_CACHE["full"] = build_nc()
    nc = _NC_CACHE["full"]
    in_maps = make_in_maps(input_, weight, bias)
    res = run_bass_kernel_spmd(
        nc, in_maps, core_ids=list(range(N_CORES)), trace=trace, **kw
    )
    outs = [np.asarray(res.results[c]["out"]) for c in range(N_CORES)]
    full = np.concatenate(outs, axis=1).reshape(S, B, F)
    return full, res


def kernel(input_, weight, bias):
    out, _ = run_spmd(input_, weight, bias, trace=False)
    return out
